# revision 2
# baseline (speedup 1.0000x reference)
"""TRN2 Bass kernel for nn_MultiHeadAttention_63977832841803 (sparse_attention).

v2: software-pipelined batches. The per-batch dataflow matches v1 (tokens on
the free axis, per-head-pair score tiles, ones-column denominator in the AV
accumulation), but the PE instruction stream is restructured so the engine
never drains at phase boundaries:

- Projections for batch b+1 are emitted INTERLEAVED into the attention groups
  of batch b (Q/K right after their SBUF tags free at (hp, c, gc=3); V spread
  over hp 1-2 into double-buffered vaug tiles), so the ACT-bound attention
  phase keeps the in-order PE fed with independent work.
- qt/mask DMAs for b+1 issue mid-batch (masks as their tags free during hp=3),
  not behind batch b's output DMAs.
- Normalization is delayed one head-pair (pb/hn for hp emitted during hp+1)
  so the den->reciprocal->broadcast chain never stalls the PE; the broadcast
  matmuls run in bf16 (fp32 matmuls cost 4x on the PE).
- All PSUM evictions are pinned to VectorE (ACT only runs the 64 exps/batch);
  an optional slice of the mask-multiplies goes to the otherwise-idle GpSimd.

Host-side preprocessing (cheap numpy): batch shard, transpose q and the masks,
pack weights per-branch as [D, H*64], cast to bf16.
"""

import numpy as np
import ml_dtypes

import concourse.bass as bass
import concourse.mybir as mybir
from concourse import bacc, tile
from concourse.bass_utils import run_bass_kernel_spmd

BF16 = mybir.dt.bfloat16
F32 = mybir.dt.float32
AF = mybir.ActivationFunctionType

H, B, G, NQ = 8, 16, 512, 512
D, KD, VD, E = 512, 64, 64, 512
NORM = 1.0 / float(np.sqrt(KD))
NCORES = 8
BLOC = B // NCORES  # batches per core
NB = ml_dtypes.bfloat16

DC, GC, HP, NCH = 4, 4, 4, 4  # d-chunks, g-chunks, head-pairs, n-chunks


def build_kernel(reps=1, bcast_mask=True, gpsimd_every=6, split_evict=True,
                 direct_out=False):  # PSUM->HBM DMA unsupported (SBUF/DRAM only)
    nc = bacc.Bacc()

    qt = nc.dram_tensor("qt", [BLOC, D, NQ], BF16, kind="ExternalInput")
    mt = nc.dram_tensor("mt", [4, BLOC, G, NQ], BF16, kind="ExternalInput")
    wq = nc.dram_tensor("wq", [4, D, H * KD], BF16, kind="ExternalInput")
    wk = nc.dram_tensor("wk", [4, D, H * KD], BF16, kind="ExternalInput")
    wv = nc.dram_tensor("wv", [4, D, H * VD], BF16, kind="ExternalInput")
    wo = nc.dram_tensor("wo", [H * VD, E], BF16, kind="ExternalInput")
    out = nc.dram_tensor("out", [BLOC, NQ, E], F32, kind="ExternalOutput")

    # PSUM: pp 2 + ps 2*2 + pav 2 = 8 banks exactly.
    with tile.TileContext(nc) as tc:
        with (
            tc.tile_pool(name="wsb", bufs=1) as wsb,        # persistent weights
            tc.tile_pool(name="qsb", bufs=2) as qsb,        # qT tiles (4 tags)
            tc.tile_pool(name="msb", bufs=1) as msb,        # mask tiles (16 tags)
            tc.tile_pool(name="proj", bufs=1) as projp,     # Q^T/K^T tiles (32 tags)
            tc.tile_pool(name="vaug", bufs=2) as vaugp,     # V tiles (16 tags, 2 bufs)
            tc.tile_pool(name="expool", bufs=5) as expool,
            tc.tile_pool(name="recp", bufs=2) as recp,      # 1/den, bf16
            tc.tile_pool(name="headsp", bufs=1) as headsp,  # hu/hn (8 tags)
            tc.tile_pool(name="osb", bufs=2) as osb,
            tc.tile_pool(name="pp", bufs=2, space="PSUM") as pp,
            tc.tile_pool(name="ps", bufs=2, space="PSUM") as ps,
            tc.tile_pool(name="pav", bufs=2, space="PSUM") as pav,
        ):
            batches = [bb for _ in range(reps) for bb in range(BLOC)]

            def load_qt(b):
                ts = []
                for dc in range(DC):
                    t = qsb.tile([128, NQ], BF16, tag=f"qt{dc}", name=f"qt{dc}")
                    nc.sync.dma_start(t[:], qt[b, 128 * dc : 128 * (dc + 1), :])
                    ts.append(t)
                return ts

            def load_mt(b, c, gc):
                t = msb.tile([128, NQ], BF16, tag=f"m{c}{gc}", name=f"m{c}{gc}")
                nc.sync.dma_start(t[:], mt[c, b, 128 * gc : 128 * (gc + 1), :])
                return t

            # batch-0 qT first: compute needs it before the weights finish
            qts_cur = load_qt(batches[0])
            wq_sb = [[wsb.tile([128, H * KD], BF16, tag=f"wq{c}{dc}", name=f"wq{c}{dc}") for dc in range(DC)] for c in range(4)]
            wk_sb = [[wsb.tile([128, H * KD], BF16, tag=f"wk{c}{dc}", name=f"wk{c}{dc}") for dc in range(DC)] for c in range(4)]
            wv_sb = [[wsb.tile([128, H * VD], BF16, tag=f"wv{c}{dc}", name=f"wv{c}{dc}") for dc in range(DC)] for c in range(4)]
            wo_sb = [wsb.tile([128, E], BF16, tag=f"wo{hc}", name=f"wo{hc}") for hc in range(4)]
            for c in range(4):
                for dc in range(DC):
                    sl = slice(128 * dc, 128 * (dc + 1))
                    nc.sync.dma_start(wq_sb[c][dc][:], wq[c, sl, :])
                    nc.sync.dma_start(wk_sb[c][dc][:], wk[c, sl, :])
                    nc.sync.dma_start(wv_sb[c][dc][:], wv[c, sl, :])
            for hc in range(4):
                nc.sync.dma_start(wo_sb[hc][:], wo[128 * hc : 128 * (hc + 1), :])
            mts_cur = [[load_mt(batches[0], c, gc) for gc in range(GC)] for c in range(4)]
            ones64r = wsb.tile([1, 64], BF16, name="ones64r")
            nc.vector.memset(ones64r[:], 1.0)

            def proj_qk(qts, c, hp, dq, dk):
                hsl = slice(128 * hp, 128 * (hp + 1))
                pq = pp.tile([128, NQ], F32, tag="pproj", name="pq")
                for dc in range(DC):
                    nc.tensor.matmul(pq[:], wq_sb[c][dc][:, hsl], qts[dc][:],
                                     start=(dc == 0), stop=(dc == DC - 1))
                tq = projp.tile([128, NQ], BF16, tag=f"q{c}{hp}", name=f"tq{c}{hp}")
                nc.vector.tensor_copy(tq[:], pq[:])
                dq[c][hp] = tq
                pk = pp.tile([128, NQ], F32, tag="pproj", name="pk")
                for dc in range(DC):
                    nc.tensor.matmul(pk[:], wk_sb[c][dc][:, hsl], qts[dc][:],
                                     start=(dc == 0), stop=(dc == DC - 1))
                tk = projp.tile([128, NQ], BF16, tag=f"k{c}{hp}", name=f"tk{c}{hp}")
                nc.vector.tensor_copy(tk[:], pk[:])
                dk[c][hp] = tk

            def proj_v(qts, c, gc, dv):
                gsl = slice(128 * gc, 128 * (gc + 1))
                pv = pp.tile([128, H * VD], F32, tag="pproj", name="pv")
                for dc in range(DC):
                    nc.tensor.matmul(pv[:], qts[dc][:, gsl], wv_sb[c][dc][:],
                                     start=(dc == 0), stop=(dc == DC - 1))
                tv = vaugp.tile([128, H * 65], BF16, tag=f"v{c}{gc}", name=f"tv{c}{gc}")
                tv3 = tv[:].rearrange("p (h v) -> p h v", v=65)
                pv3 = pv[:].rearrange("p (h v) -> p h v", v=64)
                # half the V evictions ride on ScalarE to keep VectorE under the
                # exp stream; ScalarE's strided copy costs the same as DVE's 1x
                if split_evict and (c + gc) % 2 == 0:
                    nc.scalar.copy(tv3[:, :, 0:64], pv3[:, :, :])
                else:
                    nc.vector.tensor_copy(tv3[:, :, 0:64], pv3[:, :, :])
                nc.vector.memset(tv3[:, :, 64], 1.0)
                dv[c][gc] = tv

            # prologue: projections for batch 0 (PE-serial; amortized over reps)
            tq_cur = [[None] * HP for _ in range(4)]
            tk_cur = [[None] * HP for _ in range(4)]
            tv_cur = [[None] * GC for _ in range(4)]
            for c in range(4):
                for hp in range(HP):
                    proj_qk(qts_cur, c, hp, tq_cur, tk_cur)
                for gc in range(GC):
                    proj_v(qts_cur, c, gc, tv_cur)

            for bi, b in enumerate(batches):
                nb = batches[bi + 1] if bi + 1 < len(batches) else None
                qts_next = load_qt(nb) if nb is not None else None
                tq_n = [[None] * HP for _ in range(4)]
                tk_n = [[None] * HP for _ in range(4)]
                tv_n = [[None] * GC for _ in range(4)]
                mts_n = [[None] * GC for _ in range(4)]
                rec = recp.tile([1, H * NQ], BF16, tag="rec", name="rec")
                hus = [None] * HP
                hns = [None] * HP

                def emit_norm(hp):
                    h0, h1 = 2 * hp, 2 * hp + 1
                    pb = pp.tile([128, NQ], F32, tag="pproj", name="pb")
                    nc.tensor.matmul(pb[0:64, :], ones64r[:],
                                     rec[0:1, NQ * h0 : NQ * (h0 + 1)],
                                     start=True, stop=True)
                    nc.tensor.matmul(pb[64:128, :], ones64r[:],
                                     rec[0:1, NQ * h1 : NQ * (h1 + 1)],
                                     start=True, stop=True)
                    hn = headsp.tile([128, NQ], BF16, tag=f"hn{hp}", name=f"hn{hp}")
                    nc.vector.tensor_mul(hn[:], hus[hp][:], pb[:])
                    hns[hp] = hn

                for hp in range(HP):
                    h0, h1 = 2 * hp, 2 * hp + 1
                    pa0 = pav.tile([65, NQ], F32, tag="av", name="pa0")
                    pa1 = pav.tile([65, NQ], F32, tag="av", name="pa1")
                    first, last = (0, 0), (3, GC - 1)
                    for c in range(4):
                        for gc in range(GC):
                            gsl = slice(128 * gc, 128 * (gc + 1))
                            sc = ps.tile([128, 2 * NQ], F32, tag="score", name="sc")
                            nc.tensor.matmul(sc[:, 0:NQ], tk_cur[c][hp][0:64, gsl],
                                             tq_cur[c][hp][0:64, :], start=True, stop=True)
                            nc.tensor.matmul(sc[:, NQ : 2 * NQ], tk_cur[c][hp][64:128, gsl],
                                             tq_cur[c][hp][64:128, :], start=True, stop=True)
                            ex = expool.tile([128, 2 * NQ], BF16, tag="ex", name="ex")
                            nc.scalar.activation(ex[:], sc[:], AF.Exp, scale=NORM)
                            gi = hp * 16 + c * 4 + gc
                            m = mts_cur[c][gc]
                            on_pool = bool(gpsimd_every) and (gi % gpsimd_every == gpsimd_every - 1)
                            if bcast_mask and not on_pool:
                                ex3 = ex[:].rearrange("p (two n) -> p two n", two=2)
                                m3 = m[:].rearrange("p (one n) -> p one n", one=1)
                                m3b, _ = bass.broadcast_tensor_aps(m3, ex3)
                                nc.vector.tensor_mul(ex3, ex3, m3b)
                            else:
                                eng = nc.gpsimd if on_pool else nc.vector
                                eng.tensor_mul(ex[:, 0:NQ], ex[:, 0:NQ], m[:])
                                eng.tensor_mul(ex[:, NQ : 2 * NQ], ex[:, NQ : 2 * NQ], m[:])
                            st = (c, gc) == first
                            sp = (c, gc) == last
                            nc.tensor.matmul(pa0[:], tv_cur[c][gc][:, 65 * h0 : 65 * h0 + 65],
                                             ex[:, 0:NQ], start=st, stop=sp)
                            nc.tensor.matmul(pa1[:], tv_cur[c][gc][:, 65 * h1 : 65 * h1 + 65],
                                             ex[:, NQ : 2 * NQ], start=st, stop=sp)
                            if nb is not None:
                                if gc == 3:
                                    proj_qk(qts_next, c, hp, tq_n, tk_n)
                                if hp in (1, 2):
                                    s = (hp - 1) * 16 + c * 4 + gc
                                    if s % 2 == 0:
                                        cv, gv = divmod(s // 2, GC)
                                        proj_v(qts_next, cv, gv, tv_n)
                                if hp == 3:
                                    mts_n[c][gc] = load_mt(nb, c, gc)
                    # hp tail: denominators straight from PSUM, heads eviction
                    with nc.allow_low_precision(reason="1/den in bf16; feeds a bf16 broadcast matmul, matches bf16 pipeline noise"):
                        nc.vector.reciprocal(rec[0:1, NQ * h0 : NQ * (h0 + 1)], pa0[64:65, :])
                        nc.vector.reciprocal(rec[0:1, NQ * h1 : NQ * (h1 + 1)], pa1[64:65, :])
                    hu = headsp.tile([128, NQ], BF16, tag=f"hu{hp}", name=f"hu{hp}")
                    nc.vector.tensor_copy(hu[0:64, :], pa0[0:64, :])
                    nc.vector.tensor_copy(hu[64:128, :], pa1[0:64, :])
                    hus[hp] = hu
                    if hp >= 1:
                        emit_norm(hp - 1)
                emit_norm(3)

                # ---- final W_out contraction: out[n, e] ----
                for nch in range(NCH):
                    nsl = slice(128 * nch, 128 * (nch + 1))
                    pw = pp.tile([128, E], F32, tag="pproj", name="pw")
                    for hc in range(4):
                        nc.tensor.matmul(pw[:], hns[hc][:, nsl], wo_sb[hc][:],
                                         start=(hc == 0), stop=(hc == 3))
                    if direct_out:
                        nc.sync.dma_start(out[b, nsl, :], pw[:])
                    else:
                        ot = osb.tile([128, E], F32, tag="osb", name="ot")
                        nc.vector.tensor_copy(ot[:], pw[:])
                        nc.sync.dma_start(out[b, nsl, :], ot[:])

                qts_cur, mts_cur = qts_next, mts_n
                tq_cur, tk_cur, tv_cur = tq_n, tk_n, tv_n

    nc.finalize()
    return nc


_NC_CACHE = None


def _get_nc():
    global _NC_CACHE
    if _NC_CACHE is None:
        _NC_CACHE = build_kernel()
    return _NC_CACHE


def _prep_core_inputs(q, att, grp, spd, wq_np, wk_np, wv_np, wo_np, c0):
    """Host-side shard + transpose + pack for one core's batches [c0, c0+BLOC)."""
    sl = slice(c0, c0 + BLOC)
    qt = np.ascontiguousarray(q[sl].transpose(0, 2, 1)).astype(NB)  # [BLOC, D, NQ]
    # masks in [g, n] orientation per branch: m0=sparse^T, m1=att raw, m2=att^T, m3=group^T
    mt = np.empty((4, BLOC, G, NQ), dtype=NB)
    mt[0] = spd[sl].transpose(0, 2, 1)
    mt[1] = att[sl]
    mt[2] = att[sl].transpose(0, 2, 1)
    mt[3] = grp[sl].transpose(0, 2, 1)
    return {"qt": qt, "mt": mt, "wq": wq_np, "wk": wk_np, "wv": wv_np, "wo": wo_np}


def _pack_w(ws):
    # list of 4 (H, D, Kd) -> [4, D, H*Kd] bf16
    return np.stack([w.transpose(1, 0, 2).reshape(D, -1) for w in ws]).astype(NB)


def kernel(q, att_masks, group_masks, sparse_dist_masks,
           W_query, W_K, W_V, W_Q_ps, W_K_ps, W_V_ps,
           W_Q_sp, W_K_sp, W_V_sp, W_Q_pg, W_K_pg, W_V_pg, W_out,
           _want_results=False):
    q = np.asarray(q, dtype=np.float32)
    att = np.asarray(att_masks).astype(np.float32)
    grp = np.asarray(group_masks).astype(np.float32)
    spd = np.asarray(sparse_dist_masks).astype(np.float32)

    wq_np = _pack_w([np.asarray(w, np.float32) for w in (W_query, W_Q_ps, W_Q_sp, W_Q_pg)])
    wk_np = _pack_w([np.asarray(w, np.float32) for w in (W_K, W_K_ps, W_K_sp, W_K_pg)])
    wv_np = _pack_w([np.asarray(w, np.float32) for w in (W_V, W_V_ps, W_V_sp, W_V_pg)])
    wo_np = np.asarray(W_out, np.float32).reshape(H * VD, E).astype(NB)

    in_maps = [
        _prep_core_inputs(q, att, grp, spd, wq_np, wk_np, wv_np, wo_np, BLOC * i)
        for i in range(NCORES)
    ]
    nc = _get_nc()
    res = run_bass_kernel_spmd(nc, in_maps, list(range(NCORES)))
    out = np.concatenate([res.results[i]["out"] for i in range(NCORES)], axis=0)
    if _want_results:
        return out, res
    return out


# revision 3
# speedup vs baseline: 1.5540x; 1.5540x over previous
"""TRN2 Bass kernel for nn_MultiHeadAttention_63977832841803 (sparse_attention).

v2: software-pipelined batches. The per-batch dataflow matches v1 (tokens on
the free axis, per-head-pair score tiles, ones-column denominator in the AV
accumulation), but the PE instruction stream is restructured so the engine
never drains at phase boundaries:

- Projections for batch b+1 are emitted INTERLEAVED into the attention groups
  of batch b (Q/K right after their SBUF tags free at (hp, c, gc=3); V spread
  over hp 1-2 into double-buffered vaug tiles), so the ACT-bound attention
  phase keeps the in-order PE fed with independent work.
- qt/mask DMAs for b+1 issue mid-batch (masks as their tags free during hp=3),
  not behind batch b's output DMAs.
- Normalization is delayed one head-pair (pb/hn for hp emitted during hp+1)
  so the den->reciprocal->broadcast chain never stalls the PE; the broadcast
  matmuls run in bf16 (fp32 matmuls cost 4x on the PE).
- All PSUM evictions are pinned to VectorE (ACT only runs the 64 exps/batch);
  an optional slice of the mask-multiplies goes to the otherwise-idle GpSimd.

Host-side preprocessing (cheap numpy): batch shard, transpose q and the masks,
pack weights per-branch as [D, H*64], cast to bf16.
"""

import numpy as np
import ml_dtypes

import concourse.bass as bass
import concourse.mybir as mybir
from concourse import bacc, tile
from concourse.bass_utils import run_bass_kernel_spmd

BF16 = mybir.dt.bfloat16
F32 = mybir.dt.float32
AF = mybir.ActivationFunctionType

H, B, G, NQ = 8, 16, 512, 512
D, KD, VD, E = 512, 64, 64, 512
NORM = 1.0 / float(np.sqrt(KD))
NCORES = 8
BLOC = B // NCORES  # batches per core
NB = ml_dtypes.bfloat16

DC, GC, HP, NCH = 4, 4, 4, 4  # d-chunks, g-chunks, head-pairs, n-chunks


def build_kernel(reps=1, bcast_mask=True, gpsimd_every=0, split_evict=False,
                 direct_out=False):  # PSUM->HBM DMA unsupported (SBUF/DRAM only)
    nc = bacc.Bacc()

    qt = nc.dram_tensor("qt", [BLOC, D, NQ], BF16, kind="ExternalInput")
    mt = nc.dram_tensor("mt", [4, BLOC, G, NQ], BF16, kind="ExternalInput")
    wq = nc.dram_tensor("wq", [4, D, H * KD], BF16, kind="ExternalInput")
    wk = nc.dram_tensor("wk", [4, D, H * KD], BF16, kind="ExternalInput")
    wv = nc.dram_tensor("wv", [4, D, H * VD], BF16, kind="ExternalInput")
    wo = nc.dram_tensor("wo", [H * VD, E], BF16, kind="ExternalInput")
    out = nc.dram_tensor("out", [BLOC, NQ, E], F32, kind="ExternalOutput")

    # PSUM: pp 2 + ps 2*2 + pav 2 = 8 banks exactly.
    with tile.TileContext(nc) as tc:
        with (
            tc.tile_pool(name="wsb", bufs=1) as wsb,        # persistent weights
            tc.tile_pool(name="qsb", bufs=2) as qsb,        # qT tiles (4 tags)
            tc.tile_pool(name="msb", bufs=1) as msb,        # mask tiles (16 tags)
            tc.tile_pool(name="proj", bufs=1) as projp,     # Q^T/K^T tiles (32 tags)
            tc.tile_pool(name="vaug", bufs=2) as vaugp,     # V tiles (16 tags, 2 bufs)
            tc.tile_pool(name="expool", bufs=5) as expool,
            tc.tile_pool(name="recp", bufs=2) as recp,      # 1/den, bf16
            tc.tile_pool(name="headsp", bufs=1) as headsp,  # hu/hn (8 tags)
            tc.tile_pool(name="osb", bufs=2) as osb,
            tc.tile_pool(name="pp", bufs=2, space="PSUM") as pp,
            tc.tile_pool(name="ps", bufs=2, space="PSUM") as ps,
            tc.tile_pool(name="pav", bufs=2, space="PSUM") as pav,
        ):
            batches = [bb for _ in range(reps) for bb in range(BLOC)]

            def load_qt(b):
                ts = []
                for dc in range(DC):
                    t = qsb.tile([128, NQ], BF16, tag=f"qt{dc}", name=f"qt{dc}")
                    nc.sync.dma_start(t[:], qt[b, 128 * dc : 128 * (dc + 1), :])
                    ts.append(t)
                return ts

            def load_mt(b, c, gc):
                t = msb.tile([128, NQ], BF16, tag=f"m{c}{gc}", name=f"m{c}{gc}")
                nc.sync.dma_start(t[:], mt[c, b, 128 * gc : 128 * (gc + 1), :])
                return t

            # batch-0 qT first: compute needs it before the weights finish
            qts_cur = load_qt(batches[0])
            wq_sb = [[wsb.tile([128, H * KD], BF16, tag=f"wq{c}{dc}", name=f"wq{c}{dc}") for dc in range(DC)] for c in range(4)]
            wk_sb = [[wsb.tile([128, H * KD], BF16, tag=f"wk{c}{dc}", name=f"wk{c}{dc}") for dc in range(DC)] for c in range(4)]
            wv_sb = [[wsb.tile([128, H * VD], BF16, tag=f"wv{c}{dc}", name=f"wv{c}{dc}") for dc in range(DC)] for c in range(4)]
            wo_sb = [wsb.tile([128, E], BF16, tag=f"wo{hc}", name=f"wo{hc}") for hc in range(4)]
            for c in range(4):
                for dc in range(DC):
                    sl = slice(128 * dc, 128 * (dc + 1))
                    nc.sync.dma_start(wq_sb[c][dc][:], wq[c, sl, :])
                    nc.sync.dma_start(wk_sb[c][dc][:], wk[c, sl, :])
                    nc.sync.dma_start(wv_sb[c][dc][:], wv[c, sl, :])
            for hc in range(4):
                nc.sync.dma_start(wo_sb[hc][:], wo[128 * hc : 128 * (hc + 1), :])
            mts_cur = [[load_mt(batches[0], c, gc) for gc in range(GC)] for c in range(4)]
            ones64r = wsb.tile([1, 64], BF16, name="ones64r")
            nc.vector.memset(ones64r[:], 1.0)

            def proj_qk(qts, c, hp, dq, dk):
                hsl = slice(128 * hp, 128 * (hp + 1))
                pq = pp.tile([128, NQ], F32, tag="pproj", name="pq")
                for dc in range(DC):
                    nc.tensor.matmul(pq[:], wq_sb[c][dc][:, hsl], qts[dc][:],
                                     start=(dc == 0), stop=(dc == DC - 1))
                tq = projp.tile([128, NQ], BF16, tag=f"q{c}{hp}", name=f"tq{c}{hp}")
                nc.vector.tensor_copy(tq[:], pq[:])
                dq[c][hp] = tq
                pk = pp.tile([128, NQ], F32, tag="pproj", name="pk")
                for dc in range(DC):
                    nc.tensor.matmul(pk[:], wk_sb[c][dc][:, hsl], qts[dc][:],
                                     start=(dc == 0), stop=(dc == DC - 1))
                tk = projp.tile([128, NQ], BF16, tag=f"k{c}{hp}", name=f"tk{c}{hp}")
                nc.vector.tensor_copy(tk[:], pk[:])
                dk[c][hp] = tk

            def proj_v(qts, c, gc, dv):
                gsl = slice(128 * gc, 128 * (gc + 1))
                pv = pp.tile([128, H * VD], F32, tag="pproj", name="pv")
                for dc in range(DC):
                    nc.tensor.matmul(pv[:], qts[dc][:, gsl], wv_sb[c][dc][:],
                                     start=(dc == 0), stop=(dc == DC - 1))
                tv = vaugp.tile([128, H * 65], BF16, tag=f"v{c}{gc}", name=f"tv{c}{gc}")
                tv3 = tv[:].rearrange("p (h v) -> p h v", v=65)
                pv3 = pv[:].rearrange("p (h v) -> p h v", v=64)
                # half the V evictions ride on ScalarE to keep VectorE under the
                # exp stream; ScalarE's strided copy costs the same as DVE's 1x
                if split_evict and (c + gc) % 2 == 0:
                    nc.scalar.copy(tv3[:, :, 0:64], pv3[:, :, :])
                else:
                    nc.vector.tensor_copy(tv3[:, :, 0:64], pv3[:, :, :])
                nc.vector.memset(tv3[:, :, 64], 1.0)
                dv[c][gc] = tv

            # prologue: projections for batch 0 (PE-serial; amortized over reps)
            tq_cur = [[None] * HP for _ in range(4)]
            tk_cur = [[None] * HP for _ in range(4)]
            tv_cur = [[None] * GC for _ in range(4)]
            for c in range(4):
                for hp in range(HP):
                    proj_qk(qts_cur, c, hp, tq_cur, tk_cur)
                for gc in range(GC):
                    proj_v(qts_cur, c, gc, tv_cur)

            for bi, b in enumerate(batches):
                nb = batches[bi + 1] if bi + 1 < len(batches) else None
                qts_next = load_qt(nb) if nb is not None else None
                tq_n = [[None] * HP for _ in range(4)]
                tk_n = [[None] * HP for _ in range(4)]
                tv_n = [[None] * GC for _ in range(4)]
                mts_n = [[None] * GC for _ in range(4)]
                rec = recp.tile([1, H * NQ], BF16, tag="rec", name="rec")
                hus = [None] * HP
                hns = [None] * HP

                def emit_norm(hp):
                    h0, h1 = 2 * hp, 2 * hp + 1
                    pb = pp.tile([128, NQ], F32, tag="pproj", name="pb")
                    nc.tensor.matmul(pb[0:64, :], ones64r[:],
                                     rec[0:1, NQ * h0 : NQ * (h0 + 1)],
                                     start=True, stop=True)
                    nc.tensor.matmul(pb[64:128, :], ones64r[:],
                                     rec[0:1, NQ * h1 : NQ * (h1 + 1)],
                                     start=True, stop=True)
                    hn = headsp.tile([128, NQ], BF16, tag=f"hn{hp}", name=f"hn{hp}")
                    nc.vector.tensor_mul(hn[:], hus[hp][:], pb[:])
                    hns[hp] = hn

                for hp in range(HP):
                    h0, h1 = 2 * hp, 2 * hp + 1
                    pa0 = pav.tile([65, NQ], F32, tag="av", name="pa0")
                    pa1 = pav.tile([65, NQ], F32, tag="av", name="pa1")
                    first, last = (0, 0), (3, GC - 1)
                    for c in range(4):
                        for gc in range(GC):
                            gsl = slice(128 * gc, 128 * (gc + 1))
                            sc = ps.tile([128, 2 * NQ], F32, tag="score", name="sc")
                            nc.tensor.matmul(sc[:, 0:NQ], tk_cur[c][hp][0:64, gsl],
                                             tq_cur[c][hp][0:64, :], start=True, stop=True)
                            nc.tensor.matmul(sc[:, NQ : 2 * NQ], tk_cur[c][hp][64:128, gsl],
                                             tq_cur[c][hp][64:128, :], start=True, stop=True)
                            ex = expool.tile([128, 2 * NQ], BF16, tag="ex", name="ex")
                            nc.scalar.activation(ex[:], sc[:], AF.Exp, scale=NORM)
                            gi = hp * 16 + c * 4 + gc
                            m = mts_cur[c][gc]
                            on_pool = bool(gpsimd_every) and (gi % gpsimd_every == gpsimd_every - 1)
                            if bcast_mask and not on_pool:
                                ex3 = ex[:].rearrange("p (two n) -> p two n", two=2)
                                m3 = m[:].rearrange("p (one n) -> p one n", one=1)
                                m3b, _ = bass.broadcast_tensor_aps(m3, ex3)
                                nc.vector.tensor_mul(ex3, ex3, m3b)
                            else:
                                eng = nc.gpsimd if on_pool else nc.vector
                                eng.tensor_mul(ex[:, 0:NQ], ex[:, 0:NQ], m[:])
                                eng.tensor_mul(ex[:, NQ : 2 * NQ], ex[:, NQ : 2 * NQ], m[:])
                            st = (c, gc) == first
                            sp = (c, gc) == last
                            nc.tensor.matmul(pa0[:], tv_cur[c][gc][:, 65 * h0 : 65 * h0 + 65],
                                             ex[:, 0:NQ], start=st, stop=sp)
                            nc.tensor.matmul(pa1[:], tv_cur[c][gc][:, 65 * h1 : 65 * h1 + 65],
                                             ex[:, NQ : 2 * NQ], start=st, stop=sp)
                            if nb is not None:
                                if gc == 3:
                                    proj_qk(qts_next, c, hp, tq_n, tk_n)
                                if hp in (1, 2):
                                    s = (hp - 1) * 16 + c * 4 + gc
                                    if s % 2 == 0:
                                        cv, gv = divmod(s // 2, GC)
                                        proj_v(qts_next, cv, gv, tv_n)
                                if hp == 3:
                                    mts_n[c][gc] = load_mt(nb, c, gc)
                    # hp tail: denominators straight from PSUM, heads eviction
                    with nc.allow_low_precision(reason="1/den in bf16; feeds a bf16 broadcast matmul, matches bf16 pipeline noise"):
                        nc.vector.reciprocal(rec[0:1, NQ * h0 : NQ * (h0 + 1)], pa0[64:65, :])
                        nc.vector.reciprocal(rec[0:1, NQ * h1 : NQ * (h1 + 1)], pa1[64:65, :])
                    hu = headsp.tile([128, NQ], BF16, tag=f"hu{hp}", name=f"hu{hp}")
                    nc.vector.tensor_copy(hu[0:64, :], pa0[0:64, :])
                    nc.vector.tensor_copy(hu[64:128, :], pa1[0:64, :])
                    hus[hp] = hu
                    if hp >= 1:
                        emit_norm(hp - 1)
                emit_norm(3)

                # ---- final W_out contraction: out[n, e] ----
                for nch in range(NCH):
                    nsl = slice(128 * nch, 128 * (nch + 1))
                    pw = pp.tile([128, E], F32, tag="pproj", name="pw")
                    for hc in range(4):
                        nc.tensor.matmul(pw[:], hns[hc][:, nsl], wo_sb[hc][:],
                                         start=(hc == 0), stop=(hc == 3))
                    if direct_out:
                        nc.sync.dma_start(out[b, nsl, :], pw[:])
                    else:
                        ot = osb.tile([128, E], F32, tag="osb", name="ot")
                        nc.vector.tensor_copy(ot[:], pw[:])
                        nc.sync.dma_start(out[b, nsl, :], ot[:])

                qts_cur, mts_cur = qts_next, mts_n
                tq_cur, tk_cur, tv_cur = tq_n, tk_n, tv_n

    nc.finalize()
    return nc


_NC_CACHE = None


def _get_nc():
    global _NC_CACHE
    if _NC_CACHE is None:
        _NC_CACHE = build_kernel()
    return _NC_CACHE


def _prep_core_inputs(q, att, grp, spd, wq_np, wk_np, wv_np, wo_np, c0):
    """Host-side shard + transpose + pack for one core's batches [c0, c0+BLOC)."""
    sl = slice(c0, c0 + BLOC)
    qt = np.ascontiguousarray(q[sl].transpose(0, 2, 1)).astype(NB)  # [BLOC, D, NQ]
    # masks in [g, n] orientation per branch: m0=sparse^T, m1=att raw, m2=att^T, m3=group^T
    mt = np.empty((4, BLOC, G, NQ), dtype=NB)
    mt[0] = spd[sl].transpose(0, 2, 1)
    mt[1] = att[sl]
    mt[2] = att[sl].transpose(0, 2, 1)
    mt[3] = grp[sl].transpose(0, 2, 1)
    return {"qt": qt, "mt": mt, "wq": wq_np, "wk": wk_np, "wv": wv_np, "wo": wo_np}


def _pack_w(ws):
    # list of 4 (H, D, Kd) -> [4, D, H*Kd] bf16
    return np.stack([w.transpose(1, 0, 2).reshape(D, -1) for w in ws]).astype(NB)


def kernel(q, att_masks, group_masks, sparse_dist_masks,
           W_query, W_K, W_V, W_Q_ps, W_K_ps, W_V_ps,
           W_Q_sp, W_K_sp, W_V_sp, W_Q_pg, W_K_pg, W_V_pg, W_out,
           _want_results=False):
    q = np.asarray(q, dtype=np.float32)
    att = np.asarray(att_masks).astype(np.float32)
    grp = np.asarray(group_masks).astype(np.float32)
    spd = np.asarray(sparse_dist_masks).astype(np.float32)

    wq_np = _pack_w([np.asarray(w, np.float32) for w in (W_query, W_Q_ps, W_Q_sp, W_Q_pg)])
    wk_np = _pack_w([np.asarray(w, np.float32) for w in (W_K, W_K_ps, W_K_sp, W_K_pg)])
    wv_np = _pack_w([np.asarray(w, np.float32) for w in (W_V, W_V_ps, W_V_sp, W_V_pg)])
    wo_np = np.asarray(W_out, np.float32).reshape(H * VD, E).astype(NB)

    in_maps = [
        _prep_core_inputs(q, att, grp, spd, wq_np, wk_np, wv_np, wo_np, BLOC * i)
        for i in range(NCORES)
    ]
    nc = _get_nc()
    res = run_bass_kernel_spmd(nc, in_maps, list(range(NCORES)))
    out = np.concatenate([res.results[i]["out"] for i in range(NCORES)], axis=0)
    if _want_results:
        return out, res
    return out
